# revision 21
# baseline (speedup 1.0000x reference)
"""Trainium2 Bass kernel for gnn_message_passing (nn_Mesh1_14267881357850).

Reference computation (N=200000, D_SPATIAL=64, D_STRUCT=131, D_OUT=256):
    out1 = concat(spatial, structural) @ W_comb.T + b_comb          [N, 256]
    agg  = (structural + structural[neighbour].sum(1)) * 0.25       [N, 131]
    out2 = agg @ W_agg.T + b_agg                                    [N, 256]
returns (out1, out2)

Strategy (8 cores, node-parallel, bf16 compute):
  * Nodes padded to 200704 and sharded 25088/core; `structural` is passed
    in full (bf16) to every core as the gather source (no collectives).
  * Host pre-transposes activations to feature-major bf16 and fuses them
    into a1T = [structuralT; spatialT; ones] of shape [196, 25088] so
    matmul lhsT tiles load straight from DRAM, and structural features
    0..127 form one partition-aligned tile (a1a) for the agg self-add.
  * Neighbour rows are fetched with one indirect DMA per 128 nodes per
    neighbour slot (HW limit: one offset per partition). This is the
    bottleneck: 588 SWDGE instructions x ~1.04us on the Pool engine.
  * VectorE sums the 3 neighbour rows (node-major bf16, 2x mode), PE
    transposes the sum to feature-major bf16 PSUM, VectorE adds the
    (already feature-major) self rows -> aggT in SBUF.
  * Per 128-node tile, 4 bf16 matmuls (1 cycle/row vs fp32's 4) write one
    fp32 PSUM tile [128, 512] = [out1 | out2]; Scalar/Vector copy to bf16
    SBUF; DMA to a combined DRAM output [512, 25088] bf16.
  * Biases ride as a host-provided ones-row in a1T (out1) and a memset
    ones-row in the agg K=4 tile (out2); 0.25 is folded into W_agg.
"""

import os
import sys

import numpy as np

for _p in ("/opt/trn_rl_repo", "/root/.axon_site/_ro/trn_rl_repo"):
    if os.path.isdir(_p) and _p not in sys.path:
        sys.path.append(_p)

import concourse.bacc as bacc
import concourse.bass as bass
import concourse.mybir as mybir
from concourse.bass_utils import run_bass_kernel_spmd
from concourse.masks import make_identity
from concourse.tile import TileContext

F32 = mybir.dt.float32
BF16 = mybir.dt.bfloat16
I32 = mybir.dt.int32

N = 200000
DS = 64          # spatial features
DT = 131         # structural features
DO = 256         # output features per head
NCORES = 8
GROUP = 512      # nodes per pipeline group
SUBT = GROUP // 128   # 128-node subtiles per group

NPC = 25088      # nodes per core (= 49 * 512)
NG = NPC // GROUP
NPAD = NPC * NCORES  # 200704

KA = DS + DT + 1     # 196 rows of a1T ([structural; spatial; ones])
KB = KA - 128        # 68

# exec time of the last traced run (ns), for test harnesses
last_exec_time_ns = None


def build_nc(npc=NPC, n_src=N, group=GROUP):
    """Build the Bass module for one core processing `npc` nodes."""
    ng = npc // group
    subt = group // 128
    nidx = 3 * subt              # indices per partition per group

    nc = bacc.Bacc("TRN2", target_bir_lowering=False, debug=False)
    a1T = nc.dram_tensor("a1T", [KA, npc], BF16, kind="ExternalInput")
    sfull = nc.dram_tensor("sfull", [n_src, DT], BF16, kind="ExternalInput")
    idx = nc.dram_tensor("idx", [128, ng * nidx], I32, kind="ExternalInput")
    w1 = nc.dram_tensor("w1", [KA, DO], BF16, kind="ExternalInput")
    w2 = nc.dram_tensor("w2", [DT + 1, DO], BF16, kind="ExternalInput")
    # feature-major output: rows 0..255 = out1.T, rows 256..511 = out2.T
    out = nc.dram_tensor("out", [2 * DO, npc], BF16, kind="ExternalOutput")

    with TileContext(nc) as tc:
        with (
            tc.tile_pool(name="const", bufs=1) as cpool,
            tc.tile_pool(name="work", bufs=5) as wpool,
            tc.tile_pool(name="nsums", bufs=12) as npool,
            tc.tile_pool(name="agg", bufs=5) as apool,
            tc.tile_pool(name="osb", bufs=10) as opool,
            tc.tile_pool(name="pst", bufs=2, space="PSUM") as pst,
            tc.tile_pool(name="pout", bufs=4, space="PSUM") as pout,
        ):
            # ---- constants ----
            ident = cpool.tile([128, 128], BF16)
            make_identity(nc, ident)
            w1a = cpool.tile([128, DO], BF16)
            nc.sync.dma_start(out=w1a, in_=w1[0:128, :])
            w1b = cpool.tile([KB, DO], BF16)
            nc.sync.dma_start(out=w1b, in_=w1[128:KA, :])
            w2a = cpool.tile([128, DO], BF16)
            nc.sync.dma_start(out=w2a, in_=w2[0:128, :])
            w2b = cpool.tile([4, DO], BF16)
            nc.sync.dma_start(out=w2b, in_=w2[128 : DT + 1, :])
            # split the idx preload so the first gathers start ~7us sooner:
            # a small head slice lands first, the tail loads behind it
            idx_sb = cpool.tile([128, ng * nidx], I32)
            head = 2 * nidx
            nc.sync.dma_start(out=idx_sb[:, 0:head], in_=idx[:, 0:head])
            nc.sync.dma_start(
                out=idx_sb[:, head : ng * nidx], in_=idx[:, head : ng * nidx])

            for g in range(ng):
                n0 = g * group

                # ---- loads (feature-major activations) ----
                a1a = wpool.tile([128, group], BF16, tag="a1a")
                nc.sync.dma_start(out=a1a, in_=a1T[0:128, n0 : n0 + group])
                a1b = wpool.tile([KB, group], BF16, tag="a1b")
                nc.sync.dma_start(out=a1b, in_=a1T[128:KA, n0 : n0 + group])

                # ---- indirect gathers: one DMA per (subtile, neighbour
                # slot), one row per partition (HW indirect DMA takes
                # exactly one offset per partition). All 12 land in slices
                # of ONE tile so the Pool queue pays the slot-semaphore
                # wait once per group instead of once per gather.
                gt_all = npool.tile([128, 3 * subt, DT], BF16, tag="gt")
                for b in range(subt):
                    base = (g * subt + b) * 3
                    for j in range(3):
                        nc.gpsimd.indirect_dma_start(
                            out=gt_all[:, b * 3 + j, :],
                            out_offset=None,
                            in_=sfull[:, :],
                            in_offset=bass.IndirectOffsetOnAxis(
                                ap=idx_sb[:, base + j : base + j + 1], axis=0
                            ),
                        )

                # ---- neighbour sum on VectorE, then PE transposes ----
                psA = pst.tile([128, group], BF16, tag="psA")
                psB = pst.tile([3, group], BF16, tag="psB")
                for b in range(subt):
                    nsum = npool.tile([128, DT], BF16, tag="nsum")
                    nc.vector.tensor_add(
                        out=nsum, in0=gt_all[:, b * 3, :],
                        in1=gt_all[:, b * 3 + 1, :])
                    nc.vector.tensor_add(
                        out=nsum, in0=nsum, in1=gt_all[:, b * 3 + 2, :])
                    nc.tensor.transpose(
                        psA[:, b * 128 : (b + 1) * 128],
                        nsum[:, 0:128],
                        ident,
                    )
                    nc.tensor.transpose(
                        psB[0:3, b * 128 : (b + 1) * 128],
                        nsum[:, 128:DT],
                        ident,
                    )

                # ---- aggT = nsumT + structT(self), feature-major ----
                # a1T rows: 0..130 structural, 131..194 spatial, 195 ones
                # -> struct feats 0..127 are a1a rows 0..127 (aligned),
                #    feats 128..130 are a1b rows 0..2.
                aggA = apool.tile([128, group], BF16, tag="aggA")
                nc.vector.tensor_add(out=aggA, in0=psA, in1=a1a)
                aggB = apool.tile([4, group], BF16, tag="aggB")
                # rows 0..2 overwritten below; row 3 stays 1.0 (bias row)
                nc.vector.memset(aggB[:, :], 1.0)
                nc.vector.tensor_add(
                    out=aggB[0:3, :], in0=psB[0:3, :], in1=a1b[0:3, :])

                # ---- matmuls (weights stationary, activations moving,
                # outputs feature-major) + store ----
                for c in range(2):
                    csl = slice(c * 128, (c + 1) * 128)
                    p1 = pout.tile([128, group], F32, tag="ps")
                    nc.tensor.matmul(
                        p1, lhsT=w1a[:, csl], rhs=a1a, start=True, stop=False)
                    nc.tensor.matmul(
                        p1, lhsT=w1b[:, csl], rhs=a1b, start=False, stop=True)
                    p2 = pout.tile([128, group], F32, tag="ps")
                    nc.tensor.matmul(
                        p2, lhsT=w2a[:, csl], rhs=aggA, start=True, stop=False)
                    nc.tensor.matmul(
                        p2, lhsT=w2b[:, csl], rhs=aggB, start=False, stop=True)
                    o1 = opool.tile([128, group], BF16, tag="ot")
                    nc.vector.tensor_copy(out=o1, in_=p1)
                    nc.sync.dma_start(
                        out=out[c * 128 : (c + 1) * 128, n0 : n0 + group],
                        in_=o1)
                    o2 = opool.tile([128, group], BF16, tag="ot")
                    nc.scalar.copy(out=o2, in_=p2)
                    nc.sync.dma_start(
                        out=out[DO + c * 128 : DO + (c + 1) * 128,
                                n0 : n0 + group],
                        in_=o2)
    nc.compile()
    return nc


def _to_bf16(x):
    import ml_dtypes  # noqa: PLC0415
    return np.asarray(x, np.float32).astype(ml_dtypes.bfloat16)


def prep_inputs(spatial, structural, neighbour, W_agg, b_agg, W_comb, b_comb,
                npc=NPC, ncores=NCORES, group=GROUP):
    """Host-side shard + layout transform. Returns list of per-core in_maps."""
    n = spatial.shape[0]
    npad = npc * ncores

    spatial = np.asarray(spatial, dtype=np.float32)
    structural = np.asarray(structural, dtype=np.float32)
    nbr = np.asarray(neighbour, dtype=np.int32)

    pad = npad - n
    if pad:
        spatial_p = np.concatenate(
            [spatial, np.zeros((pad, DS), np.float32)], axis=0)
        structural_p = np.concatenate(
            [structural, np.zeros((pad, DT), np.float32)], axis=0)
        nbr_p = np.concatenate([nbr, np.zeros((pad, 3), np.int32)], axis=0)
    else:
        spatial_p, structural_p, nbr_p = spatial, structural, nbr

    sfull = _to_bf16(structural)

    # w1 rows match the a1T ordering: [structural; spatial; ones-row]
    w1t = np.asarray(W_comb, np.float32).T                       # [195, 256]
    w1 = _to_bf16(np.concatenate(
        [w1t[DS : DS + DT], w1t[0:DS],
         np.asarray(b_comb, np.float32)[None, :]], axis=0))      # [196, 256]
    w2 = _to_bf16(np.concatenate(
        [0.25 * np.asarray(W_agg, np.float32).T,
         np.asarray(b_agg, np.float32)[None, :]], axis=0))       # [132, 256]

    in_maps = []
    for c in range(ncores):
        sl = slice(c * npc, (c + 1) * npc)
        a1T = np.empty((KA, npc), np.float32)
        a1T[0:DT] = structural_p[sl].T
        a1T[DT : DT + DS] = spatial_p[sl].T
        a1T[DT + DS] = 1.0
        # idx[p, (g*subt + b)*3 + j] = nbr[c*npc + g*group + b*128 + p, j]
        ngt = npc // 128
        idx = np.ascontiguousarray(
            nbr_p[sl].reshape(ngt, 128, 3)
            .transpose(1, 0, 2).reshape(128, ngt * 3))
        in_maps.append({
            "a1T": _to_bf16(a1T),
            "sfull": sfull,
            "idx": idx,
            "w1": w1,
            "w2": w2,
        })
    return in_maps


_NC_CACHE = {}


def kernel(spatial, structural, neighbour, W_agg, b_agg, W_comb, b_comb):
    global last_exec_time_ns
    key = (NPC, N, GROUP)
    if key not in _NC_CACHE:
        _NC_CACHE[key] = build_nc(*key)
    nc = _NC_CACHE[key]

    in_maps = prep_inputs(
        spatial, structural, neighbour, W_agg, b_agg, W_comb, b_comb)

    trace = bool(int(os.environ.get("KERNEL_TRACE", "0")))
    tmpdir = os.environ.get("KERNEL_TMPDIR") or None
    res = run_bass_kernel_spmd(
        nc, in_maps, core_ids=list(range(NCORES)), trace=trace, tmpdir=tmpdir)
    last_exec_time_ns = res.exec_time_ns

    comb = np.concatenate(
        [np.asarray(r["out"], np.float32) for r in res.results], axis=1)[:, :N]
    out1 = np.ascontiguousarray(comb[:DO, :].T)
    out2 = np.ascontiguousarray(comb[DO:, :].T)
    return out1, out2


# revision 22
# speedup vs baseline: 1.1745x; 1.1745x over previous
"""Trainium2 Bass kernel for gnn_message_passing (nn_Mesh1_14267881357850).

Reference computation (N=200000, D_SPATIAL=64, D_STRUCT=131, D_OUT=256):
    out1 = concat(spatial, structural) @ W_comb.T + b_comb          [N, 256]
    agg  = (structural + structural[neighbour].sum(1)) * 0.25       [N, 131]
    out2 = agg @ W_agg.T + b_agg                                    [N, 256]
returns (out1, out2)

Strategy (8 cores, node-parallel, bf16 compute):
  * Nodes padded to 200704 and sharded 25088/core; `structural` is passed
    in full (bf16) to every core as the gather source (no collectives).
  * Host pre-transposes activations to feature-major bf16 and fuses them
    into a1T = [structuralT; spatialT; ones] of shape [196, 25088] so
    matmul lhsT tiles load straight from DRAM, and structural features
    0..127 form one partition-aligned tile (a1a) for the agg self-add.
  * Neighbour rows are fetched with one indirect DMA per 128 nodes per
    neighbour slot (HW limit: one offset per partition). This is the
    bottleneck: 588 SWDGE instructions x ~1.04us on the Pool engine.
  * VectorE sums the 3 neighbour rows (node-major bf16, 2x mode), PE
    transposes the sum to feature-major bf16 PSUM, VectorE adds the
    (already feature-major) self rows -> aggT in SBUF.
  * Per 128-node tile, 4 bf16 matmuls (1 cycle/row vs fp32's 4) write one
    fp32 PSUM tile [128, 512] = [out1 | out2]; Scalar/Vector copy to bf16
    SBUF; DMA to a combined DRAM output [512, 25088] bf16.
  * Biases ride as a host-provided ones-row in a1T (out1) and a memset
    ones-row in the agg K=4 tile (out2); 0.25 is folded into W_agg.
"""

import os
import sys

import numpy as np

for _p in ("/opt/trn_rl_repo", "/root/.axon_site/_ro/trn_rl_repo"):
    if os.path.isdir(_p) and _p not in sys.path:
        sys.path.append(_p)

import concourse.bacc as bacc
import concourse.bass as bass
import concourse.mybir as mybir
from concourse.bass_utils import run_bass_kernel_spmd
from concourse.masks import make_identity
from concourse.tile import TileContext

F32 = mybir.dt.float32
BF16 = mybir.dt.bfloat16
I32 = mybir.dt.int32

N = 200000
DS = 64          # spatial features
DT = 131         # structural features
DO = 256         # output features per head
NCORES = 8
GROUP = 512      # nodes per pipeline group
SUBT = GROUP // 128   # 128-node subtiles per group

NPC = 25088      # nodes per core (= 49 * 512)
NG = NPC // GROUP
NPAD = NPC * NCORES  # 200704

KA = DS + DT + 1     # 196 rows of a1T ([structural; spatial; ones])
KB = KA - 128        # 68

# exec time of the last traced run (ns), for test harnesses
last_exec_time_ns = None


def build_nc(npc=NPC, n_src=N, group=GROUP):
    """Build the Bass module for one core processing `npc` nodes."""
    ng = npc // group
    subt = group // 128
    nidx = 3 * subt              # indices per partition per group

    nc = bacc.Bacc("TRN2", target_bir_lowering=False, debug=False)
    a1T = nc.dram_tensor("a1T", [KA, npc], BF16, kind="ExternalInput")
    sfull = nc.dram_tensor("sfull", [n_src, DT], BF16, kind="ExternalInput")
    idx = nc.dram_tensor("idx", [128, ng * nidx], I32, kind="ExternalInput")
    w1 = nc.dram_tensor("w1", [KA, DO], BF16, kind="ExternalInput")
    w2 = nc.dram_tensor("w2", [DT + 1, DO], BF16, kind="ExternalInput")
    # feature-major output: rows 0..255 = out1.T, rows 256..511 = out2.T
    out = nc.dram_tensor("out", [2 * DO, npc], BF16, kind="ExternalOutput")

    with TileContext(nc) as tc:
        with (
            tc.tile_pool(name="const", bufs=1) as cpool,
            tc.tile_pool(name="work", bufs=5) as wpool,
            tc.tile_pool(name="nsums", bufs=12) as npool,
            tc.tile_pool(name="agg", bufs=5) as apool,
            tc.tile_pool(name="osb", bufs=10) as opool,
            tc.tile_pool(name="pst", bufs=2, space="PSUM") as pst,
            tc.tile_pool(name="pout", bufs=4, space="PSUM") as pout,
        ):
            # ---- constants ----
            ident = cpool.tile([128, 128], BF16)
            make_identity(nc, ident)
            w1a = cpool.tile([128, DO], BF16)
            nc.sync.dma_start(out=w1a, in_=w1[0:128, :])
            w1b = cpool.tile([KB, DO], BF16)
            nc.sync.dma_start(out=w1b, in_=w1[128:KA, :])
            w2a = cpool.tile([128, DO], BF16)
            nc.sync.dma_start(out=w2a, in_=w2[0:128, :])
            w2b = cpool.tile([4, DO], BF16)
            nc.sync.dma_start(out=w2b, in_=w2[128 : DT + 1, :])
            idx_sb = cpool.tile([128, ng * nidx], I32)
            nc.sync.dma_start(out=idx_sb, in_=idx[:, :])

            for g in range(ng):
                n0 = g * group

                # ---- loads (feature-major activations) ----
                a1a = wpool.tile([128, group], BF16, tag="a1a")
                nc.sync.dma_start(out=a1a, in_=a1T[0:128, n0 : n0 + group])
                a1b = wpool.tile([KB, group], BF16, tag="a1b")
                nc.sync.dma_start(out=a1b, in_=a1T[128:KA, n0 : n0 + group])

                # ---- indirect gathers: one DMA per (subtile, neighbour
                # slot), one row per partition (HW indirect DMA takes
                # exactly one offset per partition). All 12 land in slices
                # of ONE tile so the Pool queue pays the slot-semaphore
                # wait once per group instead of once per gather.
                gt_all = npool.tile([128, 3 * subt, DT], BF16, tag="gt")
                for b in range(subt):
                    base = (g * subt + b) * 3
                    for j in range(3):
                        nc.gpsimd.indirect_dma_start(
                            out=gt_all[:, b * 3 + j, :],
                            out_offset=None,
                            in_=sfull[:, :],
                            in_offset=bass.IndirectOffsetOnAxis(
                                ap=idx_sb[:, base + j : base + j + 1], axis=0
                            ),
                        )

                # ---- neighbour sum on VectorE, then PE transposes ----
                psA = pst.tile([128, group], BF16, tag="psA")
                psB = pst.tile([3, group], BF16, tag="psB")
                for b in range(subt):
                    nsum = npool.tile([128, DT], BF16, tag="nsum")
                    nc.vector.tensor_add(
                        out=nsum, in0=gt_all[:, b * 3, :],
                        in1=gt_all[:, b * 3 + 1, :])
                    nc.vector.tensor_add(
                        out=nsum, in0=nsum, in1=gt_all[:, b * 3 + 2, :])
                    nc.tensor.transpose(
                        psA[:, b * 128 : (b + 1) * 128],
                        nsum[:, 0:128],
                        ident,
                    )
                    nc.tensor.transpose(
                        psB[0:3, b * 128 : (b + 1) * 128],
                        nsum[:, 128:DT],
                        ident,
                    )

                # ---- aggT = nsumT + structT(self), feature-major ----
                # a1T rows: 0..130 structural, 131..194 spatial, 195 ones
                # -> struct feats 0..127 are a1a rows 0..127 (aligned),
                #    feats 128..130 are a1b rows 0..2.
                aggA = apool.tile([128, group], BF16, tag="aggA")
                nc.vector.tensor_add(out=aggA, in0=psA, in1=a1a)
                aggB = apool.tile([4, group], BF16, tag="aggB")
                # rows 0..2 overwritten below; row 3 stays 1.0 (bias row)
                nc.vector.memset(aggB[:, :], 1.0)
                nc.vector.tensor_add(
                    out=aggB[0:3, :], in0=psB[0:3, :], in1=a1b[0:3, :])

                # ---- matmuls (weights stationary, activations moving,
                # outputs feature-major) + store ----
                for c in range(2):
                    csl = slice(c * 128, (c + 1) * 128)
                    p1 = pout.tile([128, group], F32, tag="ps")
                    nc.tensor.matmul(
                        p1, lhsT=w1a[:, csl], rhs=a1a, start=True, stop=False)
                    nc.tensor.matmul(
                        p1, lhsT=w1b[:, csl], rhs=a1b, start=False, stop=True)
                    p2 = pout.tile([128, group], F32, tag="ps")
                    nc.tensor.matmul(
                        p2, lhsT=w2a[:, csl], rhs=aggA, start=True, stop=False)
                    nc.tensor.matmul(
                        p2, lhsT=w2b[:, csl], rhs=aggB, start=False, stop=True)
                    o1 = opool.tile([128, group], BF16, tag="ot")
                    nc.vector.tensor_copy(out=o1, in_=p1)
                    nc.sync.dma_start(
                        out=out[c * 128 : (c + 1) * 128, n0 : n0 + group],
                        in_=o1)
                    o2 = opool.tile([128, group], BF16, tag="ot")
                    nc.scalar.copy(out=o2, in_=p2)
                    nc.sync.dma_start(
                        out=out[DO + c * 128 : DO + (c + 1) * 128,
                                n0 : n0 + group],
                        in_=o2)
    nc.compile()
    return nc


def _to_bf16(x):
    import ml_dtypes  # noqa: PLC0415
    return np.asarray(x, np.float32).astype(ml_dtypes.bfloat16)


def prep_inputs(spatial, structural, neighbour, W_agg, b_agg, W_comb, b_comb,
                npc=NPC, ncores=NCORES, group=GROUP):
    """Host-side shard + layout transform. Returns list of per-core in_maps."""
    n = spatial.shape[0]
    npad = npc * ncores

    spatial = np.asarray(spatial, dtype=np.float32)
    structural = np.asarray(structural, dtype=np.float32)
    nbr = np.asarray(neighbour, dtype=np.int32)

    pad = npad - n
    if pad:
        spatial_p = np.concatenate(
            [spatial, np.zeros((pad, DS), np.float32)], axis=0)
        structural_p = np.concatenate(
            [structural, np.zeros((pad, DT), np.float32)], axis=0)
        nbr_p = np.concatenate([nbr, np.zeros((pad, 3), np.int32)], axis=0)
    else:
        spatial_p, structural_p, nbr_p = spatial, structural, nbr

    sfull = _to_bf16(structural)

    # w1 rows match the a1T ordering: [structural; spatial; ones-row]
    w1t = np.asarray(W_comb, np.float32).T                       # [195, 256]
    w1 = _to_bf16(np.concatenate(
        [w1t[DS : DS + DT], w1t[0:DS],
         np.asarray(b_comb, np.float32)[None, :]], axis=0))      # [196, 256]
    w2 = _to_bf16(np.concatenate(
        [0.25 * np.asarray(W_agg, np.float32).T,
         np.asarray(b_agg, np.float32)[None, :]], axis=0))       # [132, 256]

    in_maps = []
    for c in range(ncores):
        sl = slice(c * npc, (c + 1) * npc)
        a1T = np.empty((KA, npc), np.float32)
        a1T[0:DT] = structural_p[sl].T
        a1T[DT : DT + DS] = spatial_p[sl].T
        a1T[DT + DS] = 1.0
        # idx[p, (g*subt + b)*3 + j] = nbr[c*npc + g*group + b*128 + p, j]
        ngt = npc // 128
        idx = np.ascontiguousarray(
            nbr_p[sl].reshape(ngt, 128, 3)
            .transpose(1, 0, 2).reshape(128, ngt * 3))
        in_maps.append({
            "a1T": _to_bf16(a1T),
            "sfull": sfull,
            "idx": idx,
            "w1": w1,
            "w2": w2,
        })
    return in_maps


_NC_CACHE = {}


def kernel(spatial, structural, neighbour, W_agg, b_agg, W_comb, b_comb):
    global last_exec_time_ns
    key = (NPC, N, GROUP)
    if key not in _NC_CACHE:
        _NC_CACHE[key] = build_nc(*key)
    nc = _NC_CACHE[key]

    in_maps = prep_inputs(
        spatial, structural, neighbour, W_agg, b_agg, W_comb, b_comb)

    trace = bool(int(os.environ.get("KERNEL_TRACE", "0")))
    tmpdir = os.environ.get("KERNEL_TMPDIR") or None
    res = run_bass_kernel_spmd(
        nc, in_maps, core_ids=list(range(NCORES)), trace=trace, tmpdir=tmpdir)
    last_exec_time_ns = res.exec_time_ns

    comb = np.concatenate(
        [np.asarray(r["out"], np.float32) for r in res.results], axis=1)[:, :N]
    out1 = np.ascontiguousarray(comb[:DO, :].T)
    out2 = np.ascontiguousarray(comb[DO:, :].T)
    return out1, out2


# revision 24
# speedup vs baseline: 1.1758x; 1.0011x over previous
"""Trainium2 Bass kernel for gnn_message_passing (nn_Mesh1_14267881357850).

Reference computation (N=200000, D_SPATIAL=64, D_STRUCT=131, D_OUT=256):
    out1 = concat(spatial, structural) @ W_comb.T + b_comb          [N, 256]
    agg  = (structural + structural[neighbour].sum(1)) * 0.25       [N, 131]
    out2 = agg @ W_agg.T + b_agg                                    [N, 256]
returns (out1, out2)

Strategy (8 cores, node-parallel, bf16 compute):
  * Nodes padded to 200704 and sharded 25088/core; `structural` is passed
    in full (bf16) to every core as the gather source (no collectives).
  * Host pre-transposes activations to feature-major bf16 and fuses them
    into a1T = [structuralT; spatialT; ones] of shape [196, 25088] so
    matmul lhsT tiles load straight from DRAM, and structural features
    0..127 form one partition-aligned tile (a1a) for the agg self-add.
  * Neighbour rows are fetched with one indirect DMA per 128 nodes per
    neighbour slot (HW limit: one offset per partition). This is the
    bottleneck: 588 SWDGE instructions x ~1.04us on the Pool engine.
  * VectorE sums the 3 neighbour rows (node-major bf16, 2x mode), PE
    transposes the sum to feature-major bf16 PSUM, VectorE adds the
    (already feature-major) self rows -> aggT in SBUF.
  * Per 128-node tile, 4 bf16 matmuls (1 cycle/row vs fp32's 4) write one
    fp32 PSUM tile [128, 512] = [out1 | out2]; Scalar/Vector copy to bf16
    SBUF; DMA to a combined DRAM output [512, 25088] bf16.
  * Biases ride as a host-provided ones-row in a1T (out1) and a memset
    ones-row in the agg K=4 tile (out2); 0.25 is folded into W_agg.
"""

import os
import sys

import numpy as np

for _p in ("/opt/trn_rl_repo", "/root/.axon_site/_ro/trn_rl_repo"):
    if os.path.isdir(_p) and _p not in sys.path:
        sys.path.append(_p)

import concourse.bacc as bacc
import concourse.bass as bass
import concourse.mybir as mybir
from concourse.bass_utils import run_bass_kernel_spmd
from concourse.masks import make_identity
from concourse.tile import TileContext

F32 = mybir.dt.float32
BF16 = mybir.dt.bfloat16
I32 = mybir.dt.int32

N = 200000
DS = 64          # spatial features
DT = 131         # structural features
DO = 256         # output features per head
NCORES = 8
GROUP = 512      # nodes per pipeline group
SUBT = GROUP // 128   # 128-node subtiles per group

NPC = 25088      # nodes per core (= 49 * 512)
NG = NPC // GROUP
NPAD = NPC * NCORES  # 200704

KA = DS + DT + 1     # 196 rows of a1T ([structural; spatial; ones])
KB = KA - 128        # 68

# exec time of the last traced run (ns), for test harnesses
last_exec_time_ns = None


def build_nc(npc=NPC, n_src=N, group=GROUP):
    """Build the Bass module for one core processing `npc` nodes."""
    ng = npc // group
    subt = group // 128
    nidx = 3 * subt              # indices per partition per group

    nc = bacc.Bacc("TRN2", target_bir_lowering=False, debug=False)
    a1T = nc.dram_tensor("a1T", [KA, npc], BF16, kind="ExternalInput")
    sfull = nc.dram_tensor("sfull", [n_src, DT], BF16, kind="ExternalInput")
    idx = nc.dram_tensor("idx", [128, ng * nidx], I32, kind="ExternalInput")
    w1 = nc.dram_tensor("w1", [KA, DO], BF16, kind="ExternalInput")
    w2 = nc.dram_tensor("w2", [DT + 1, DO], BF16, kind="ExternalInput")
    # feature-major output: rows 0..255 = out1.T, rows 256..511 = out2.T
    out = nc.dram_tensor("out", [2 * DO, npc], BF16, kind="ExternalOutput")

    with TileContext(nc) as tc:
        with (
            tc.tile_pool(name="const", bufs=1) as cpool,
            tc.tile_pool(name="work", bufs=5) as wpool,
            tc.tile_pool(name="nsums", bufs=12) as npool,
            tc.tile_pool(name="agg", bufs=5) as apool,
            tc.tile_pool(name="osb", bufs=10) as opool,
            tc.tile_pool(name="pst", bufs=2, space="PSUM") as pst,
            tc.tile_pool(name="pout", bufs=4, space="PSUM") as pout,
        ):
            # ---- constants ----
            ident = cpool.tile([128, 128], BF16)
            make_identity(nc, ident)
            w1a = cpool.tile([128, DO], BF16)
            nc.sync.dma_start(out=w1a, in_=w1[0:128, :])
            w1b = cpool.tile([KB, DO], BF16)
            nc.sync.dma_start(out=w1b, in_=w1[128:KA, :])
            w2a = cpool.tile([128, DO], BF16)
            nc.sync.dma_start(out=w2a, in_=w2[0:128, :])
            w2b = cpool.tile([4, DO], BF16)
            nc.sync.dma_start(out=w2b, in_=w2[128 : DT + 1, :])
            idx_sb = cpool.tile([128, ng * nidx], I32)
            nc.sync.dma_start(out=idx_sb, in_=idx[:, :])

            for g in range(ng):
                n0 = g * group

                # ---- loads (feature-major activations) ----
                a1a = wpool.tile([128, group], BF16, tag="a1a")
                nc.sync.dma_start(out=a1a, in_=a1T[0:128, n0 : n0 + group])
                a1b = wpool.tile([KB, group], BF16, tag="a1b")
                nc.sync.dma_start(out=a1b, in_=a1T[128:KA, n0 : n0 + group])

                # ---- indirect gathers: one DMA per (subtile, neighbour
                # slot), one row per partition (HW indirect DMA takes
                # exactly one offset per partition). All 12 land in slices
                # of ONE tile so the Pool queue pays the slot-semaphore
                # wait once per group instead of once per gather.
                gt_all = npool.tile([128, 3 * subt, DT], BF16, tag="gt")
                for b in range(subt):
                    base = (g * subt + b) * 3
                    for j in range(3):
                        nc.gpsimd.indirect_dma_start(
                            out=gt_all[:, b * 3 + j, :],
                            out_offset=None,
                            in_=sfull[:, :],
                            in_offset=bass.IndirectOffsetOnAxis(
                                ap=idx_sb[:, base + j : base + j + 1], axis=0
                            ),
                        )

                # ---- neighbour sum on VectorE, then PE transposes ----
                psA = pst.tile([128, group], BF16, tag="psA")
                psB = pst.tile([3, group], BF16, tag="psB")
                for b in range(subt):
                    nsum = npool.tile([128, DT], BF16, tag="nsum")
                    nc.vector.tensor_add(
                        out=nsum, in0=gt_all[:, b * 3, :],
                        in1=gt_all[:, b * 3 + 1, :])
                    nc.vector.tensor_add(
                        out=nsum, in0=nsum, in1=gt_all[:, b * 3 + 2, :])
                    nc.tensor.transpose(
                        psA[:, b * 128 : (b + 1) * 128],
                        nsum[:, 0:128],
                        ident,
                    )
                    nc.tensor.transpose(
                        psB[0:3, b * 128 : (b + 1) * 128],
                        nsum[:, 128:DT],
                        ident,
                    )

                # ---- aggT = nsumT + structT(self), feature-major ----
                # a1T rows: 0..130 structural, 131..194 spatial, 195 ones
                # -> struct feats 0..127 are a1a rows 0..127 (aligned),
                #    feats 128..130 are a1b rows 0..2.
                aggA = apool.tile([128, group], BF16, tag="aggA")
                nc.vector.tensor_add(out=aggA, in0=psA, in1=a1a)
                aggB = apool.tile([4, group], BF16, tag="aggB")
                # rows 0..2 overwritten below; row 3 stays 1.0 (bias row)
                nc.vector.memset(aggB[:, :], 1.0)
                nc.vector.tensor_add(
                    out=aggB[0:3, :], in0=psB[0:3, :], in1=a1b[0:3, :])

                # ---- matmuls (weights stationary, activations moving,
                # outputs feature-major) + store ----
                for c in range(2):
                    csl = slice(c * 128, (c + 1) * 128)
                    p1 = pout.tile([128, group], F32, tag="ps")
                    nc.tensor.matmul(
                        p1, lhsT=w1a[:, csl], rhs=a1a, start=True, stop=False)
                    nc.tensor.matmul(
                        p1, lhsT=w1b[:, csl], rhs=a1b, start=False, stop=True)
                    p2 = pout.tile([128, group], F32, tag="ps")
                    nc.tensor.matmul(
                        p2, lhsT=w2a[:, csl], rhs=aggA, start=True, stop=False)
                    nc.tensor.matmul(
                        p2, lhsT=w2b[:, csl], rhs=aggB, start=False, stop=True)
                    o1 = opool.tile([128, group], BF16, tag="ot")
                    nc.vector.tensor_copy(out=o1, in_=p1)
                    nc.sync.dma_start(
                        out=out[c * 128 : (c + 1) * 128, n0 : n0 + group],
                        in_=o1)
                    o2 = opool.tile([128, group], BF16, tag="ot")
                    nc.scalar.copy(out=o2, in_=p2)
                    nc.sync.dma_start(
                        out=out[DO + c * 128 : DO + (c + 1) * 128,
                                n0 : n0 + group],
                        in_=o2)
    nc.compile()
    return nc


def _to_bf16(x):
    import ml_dtypes  # noqa: PLC0415
    return np.asarray(x, np.float32).astype(ml_dtypes.bfloat16)


def prep_inputs(spatial, structural, neighbour, W_agg, b_agg, W_comb, b_comb,
                npc=NPC, ncores=NCORES, group=GROUP):
    """Host-side shard + layout transform. Returns list of per-core in_maps."""
    n = spatial.shape[0]
    npad = npc * ncores

    spatial = np.asarray(spatial, dtype=np.float32)
    structural = np.asarray(structural, dtype=np.float32)
    nbr = np.asarray(neighbour, dtype=np.int32)

    pad = npad - n
    if pad:
        spatial_p = np.concatenate(
            [spatial, np.zeros((pad, DS), np.float32)], axis=0)
        structural_p = np.concatenate(
            [structural, np.zeros((pad, DT), np.float32)], axis=0)
        nbr_p = np.concatenate([nbr, np.zeros((pad, 3), np.int32)], axis=0)
    else:
        spatial_p, structural_p, nbr_p = spatial, structural, nbr

    sfull = _to_bf16(structural)

    # w1 rows match the a1T ordering: [structural; spatial; ones-row]
    w1t = np.asarray(W_comb, np.float32).T                       # [195, 256]
    w1 = _to_bf16(np.concatenate(
        [w1t[DS : DS + DT], w1t[0:DS],
         np.asarray(b_comb, np.float32)[None, :]], axis=0))      # [196, 256]
    w2 = _to_bf16(np.concatenate(
        [0.25 * np.asarray(W_agg, np.float32).T,
         np.asarray(b_agg, np.float32)[None, :]], axis=0))       # [132, 256]

    in_maps = []
    for c in range(ncores):
        sl = slice(c * npc, (c + 1) * npc)
        a1T = np.empty((KA, npc), np.float32)
        a1T[0:DT] = structural_p[sl].T
        a1T[DT : DT + DS] = spatial_p[sl].T
        a1T[DT + DS] = 1.0
        # idx[p, (g*subt + b)*3 + j] = nbr[c*npc + g*group + b*128 + p, j]
        ngt = npc // 128
        idx = np.ascontiguousarray(
            nbr_p[sl].reshape(ngt, 128, 3)
            .transpose(1, 0, 2).reshape(128, ngt * 3))
        in_maps.append({
            "a1T": _to_bf16(a1T),
            "sfull": sfull,
            "idx": idx,
            "w1": w1,
            "w2": w2,
        })
    return in_maps


_NC_CACHE = {}


def kernel(spatial, structural, neighbour, W_agg, b_agg, W_comb, b_comb):
    global last_exec_time_ns
    key = (NPC, N, GROUP)
    if key not in _NC_CACHE:
        _NC_CACHE[key] = build_nc(*key)
    nc = _NC_CACHE[key]

    in_maps = prep_inputs(
        spatial, structural, neighbour, W_agg, b_agg, W_comb, b_comb)

    trace = bool(int(os.environ.get("KERNEL_TRACE", "0")))
    tmpdir = os.environ.get("KERNEL_TMPDIR") or None
    res = run_bass_kernel_spmd(
        nc, in_maps, core_ids=list(range(NCORES)), trace=trace, tmpdir=tmpdir)
    last_exec_time_ns = res.exec_time_ns

    comb = np.concatenate(
        [np.asarray(r["out"], np.float32) for r in res.results], axis=1)[:, :N]
    out1 = np.ascontiguousarray(comb[:DO, :].T)
    out2 = np.ascontiguousarray(comb[DO:, :].T)
    return out1, out2


# revision 27
# speedup vs baseline: 1.1772x; 1.0012x over previous
"""Trainium2 Bass kernel for gnn_message_passing (nn_Mesh1_14267881357850).

Reference computation (N=200000, D_SPATIAL=64, D_STRUCT=131, D_OUT=256):
    out1 = concat(spatial, structural) @ W_comb.T + b_comb          [N, 256]
    agg  = (structural + structural[neighbour].sum(1)) * 0.25       [N, 131]
    out2 = agg @ W_agg.T + b_agg                                    [N, 256]
returns (out1, out2)

Strategy (8 cores, node-parallel, bf16 compute):
  * Nodes padded to 200704 and sharded 25088/core; `structural` is passed
    in full (bf16) to every core as the gather source (no collectives).
  * Host pre-transposes activations to feature-major bf16 and fuses them
    into a1T = [structuralT; spatialT; ones] of shape [196, 25088] so
    matmul lhsT tiles load straight from DRAM, and structural features
    0..127 form one partition-aligned tile (a1a) for the agg self-add.
  * Neighbour rows are fetched with one indirect DMA per 128 nodes per
    neighbour slot (HW limit: one offset per partition). This is the
    bottleneck: 588 SWDGE instructions x ~1.04us on the Pool engine.
  * VectorE sums the 3 neighbour rows (node-major bf16, 2x mode), PE
    transposes the sum to feature-major bf16 PSUM, VectorE adds the
    (already feature-major) self rows -> aggT in SBUF.
  * Per 128-node tile, 4 bf16 matmuls (1 cycle/row vs fp32's 4) write one
    fp32 PSUM tile [128, 512] = [out1 | out2]; Scalar/Vector copy to bf16
    SBUF; DMA to a combined DRAM output [512, 25088] bf16.
  * Biases ride as a host-provided ones-row in a1T (out1) and a memset
    ones-row in the agg K=4 tile (out2); 0.25 is folded into W_agg.
"""

import os
import sys

import numpy as np

for _p in ("/opt/trn_rl_repo", "/root/.axon_site/_ro/trn_rl_repo"):
    if os.path.isdir(_p) and _p not in sys.path:
        sys.path.append(_p)

import concourse.bacc as bacc
import concourse.bass as bass
import concourse.mybir as mybir
from concourse.bass_utils import run_bass_kernel_spmd
from concourse.masks import make_identity
from concourse.tile import TileContext

F32 = mybir.dt.float32
BF16 = mybir.dt.bfloat16
I32 = mybir.dt.int32

N = 200000
DS = 64          # spatial features
DT = 131         # structural features
DO = 256         # output features per head
NCORES = 8
GROUP = 512      # nodes per pipeline group
SUBT = GROUP // 128   # 128-node subtiles per group

NPC = 25088      # nodes per core (= 49 * 512)
NG = NPC // GROUP
NPAD = NPC * NCORES  # 200704

KA = DS + DT + 1     # 196 rows of a1T ([structural; spatial; ones])
KB = KA - 128        # 68

# exec time of the last traced run (ns), for test harnesses
last_exec_time_ns = None


def build_nc(npc=NPC, n_src=N, group=GROUP):
    """Build the Bass module for one core processing `npc` nodes."""
    ng = npc // group
    subt = group // 128
    nidx = 3 * subt              # indices per partition per group

    nc = bacc.Bacc("TRN2", target_bir_lowering=False, debug=False)
    a1T = nc.dram_tensor("a1T", [KA, npc], BF16, kind="ExternalInput")
    sfull = nc.dram_tensor("sfull", [n_src, DT], BF16, kind="ExternalInput")
    idx = nc.dram_tensor("idx", [128, ng * nidx], I32, kind="ExternalInput")
    w1 = nc.dram_tensor("w1", [KA, DO], BF16, kind="ExternalInput")
    w2 = nc.dram_tensor("w2", [DT + 1, DO], BF16, kind="ExternalInput")
    # feature-major output: rows 0..255 = out1.T, rows 256..511 = out2.T
    out = nc.dram_tensor("out", [2 * DO, npc], BF16, kind="ExternalOutput")

    with TileContext(nc) as tc:
        with (
            tc.tile_pool(name="const", bufs=1) as cpool,
            tc.tile_pool(name="work", bufs=5) as wpool,
            tc.tile_pool(name="nsums", bufs=12) as npool,
            tc.tile_pool(name="agg", bufs=5) as apool,
            tc.tile_pool(name="osb", bufs=10) as opool,
            tc.tile_pool(name="pst", bufs=2, space="PSUM") as pst,
            tc.tile_pool(name="pout", bufs=4, space="PSUM") as pout,
        ):
            # ---- constants ----
            ident = cpool.tile([128, 128], BF16)
            make_identity(nc, ident)
            w1a = cpool.tile([128, DO], BF16)
            nc.sync.dma_start(out=w1a, in_=w1[0:128, :])
            w1b = cpool.tile([KB, DO], BF16)
            nc.sync.dma_start(out=w1b, in_=w1[128:KA, :])
            w2a = cpool.tile([128, DO], BF16)
            nc.sync.dma_start(out=w2a, in_=w2[0:128, :])
            w2b = cpool.tile([4, DO], BF16)
            nc.sync.dma_start(out=w2b, in_=w2[128 : DT + 1, :])
            idx_sb = cpool.tile([128, ng * nidx], I32)
            nc.sync.dma_start(out=idx_sb, in_=idx[:, :])

            for g in range(ng):
                n0 = g * group

                # ---- loads (feature-major activations) ----
                a1a = wpool.tile([128, group], BF16, tag="a1a")
                nc.sync.dma_start(out=a1a, in_=a1T[0:128, n0 : n0 + group])
                a1b = wpool.tile([KB, group], BF16, tag="a1b")
                nc.sync.dma_start(out=a1b, in_=a1T[128:KA, n0 : n0 + group])

                # ---- indirect gathers: one DMA per (subtile, neighbour
                # slot), one row per partition (HW indirect DMA takes
                # exactly one offset per partition). All 12 land in slices
                # of ONE tile so the Pool queue pays the slot-semaphore
                # wait once per group instead of once per gather.
                gt_all = npool.tile([128, 3 * subt, DT], BF16, tag="gt")
                for b in range(subt):
                    base = (g * subt + b) * 3
                    for j in range(3):
                        nc.gpsimd.indirect_dma_start(
                            out=gt_all[:, b * 3 + j, :],
                            out_offset=None,
                            in_=sfull[:, :],
                            in_offset=bass.IndirectOffsetOnAxis(
                                ap=idx_sb[:, base + j : base + j + 1], axis=0
                            ),
                        )

                # ---- neighbour sum on VectorE, then PE transposes ----
                psA = pst.tile([128, group], BF16, tag="psA")
                psB = pst.tile([3, group], BF16, tag="psB")
                for b in range(subt):
                    nsum = npool.tile([128, DT], BF16, tag="nsum")
                    nc.vector.tensor_add(
                        out=nsum, in0=gt_all[:, b * 3, :],
                        in1=gt_all[:, b * 3 + 1, :])
                    nc.vector.tensor_add(
                        out=nsum, in0=nsum, in1=gt_all[:, b * 3 + 2, :])
                    nc.tensor.transpose(
                        psA[:, b * 128 : (b + 1) * 128],
                        nsum[:, 0:128],
                        ident,
                    )
                    nc.tensor.transpose(
                        psB[0:3, b * 128 : (b + 1) * 128],
                        nsum[:, 128:DT],
                        ident,
                    )

                # ---- aggT = nsumT + structT(self), feature-major ----
                # a1T rows: 0..130 structural, 131..194 spatial, 195 ones
                # -> struct feats 0..127 are a1a rows 0..127 (aligned),
                #    feats 128..130 are a1b rows 0..2.
                aggA = apool.tile([128, group], BF16, tag="aggA")
                nc.vector.tensor_add(out=aggA, in0=psA, in1=a1a)
                aggB = apool.tile([4, group], BF16, tag="aggB")
                # rows 0..2 overwritten below; row 3 stays 1.0 (bias row)
                nc.vector.memset(aggB[:, :], 1.0)
                nc.vector.tensor_add(
                    out=aggB[0:3, :], in0=psB[0:3, :], in1=a1b[0:3, :])

                # ---- matmuls (weights stationary, activations moving,
                # outputs feature-major) + store ----
                for c in range(2):
                    csl = slice(c * 128, (c + 1) * 128)
                    p1 = pout.tile([128, group], F32, tag="ps")
                    nc.tensor.matmul(
                        p1, lhsT=w1a[:, csl], rhs=a1a, start=True, stop=False)
                    nc.tensor.matmul(
                        p1, lhsT=w1b[:, csl], rhs=a1b, start=False, stop=True)
                    p2 = pout.tile([128, group], F32, tag="ps")
                    nc.tensor.matmul(
                        p2, lhsT=w2a[:, csl], rhs=aggA, start=True, stop=False)
                    nc.tensor.matmul(
                        p2, lhsT=w2b[:, csl], rhs=aggB, start=False, stop=True)
                    o1 = opool.tile([128, group], BF16, tag="ot")
                    nc.vector.tensor_copy(out=o1, in_=p1)
                    nc.scalar.dma_start(
                        out=out[c * 128 : (c + 1) * 128, n0 : n0 + group],
                        in_=o1)
                    o2 = opool.tile([128, group], BF16, tag="ot")
                    nc.scalar.copy(out=o2, in_=p2)
                    nc.sync.dma_start(
                        out=out[DO + c * 128 : DO + (c + 1) * 128,
                                n0 : n0 + group],
                        in_=o2)
    nc.compile()
    return nc


def _to_bf16(x):
    import ml_dtypes  # noqa: PLC0415
    return np.asarray(x, np.float32).astype(ml_dtypes.bfloat16)


def prep_inputs(spatial, structural, neighbour, W_agg, b_agg, W_comb, b_comb,
                npc=NPC, ncores=NCORES, group=GROUP):
    """Host-side shard + layout transform. Returns list of per-core in_maps."""
    n = spatial.shape[0]
    npad = npc * ncores

    spatial = np.asarray(spatial, dtype=np.float32)
    structural = np.asarray(structural, dtype=np.float32)
    nbr = np.asarray(neighbour, dtype=np.int32)

    pad = npad - n
    if pad:
        spatial_p = np.concatenate(
            [spatial, np.zeros((pad, DS), np.float32)], axis=0)
        structural_p = np.concatenate(
            [structural, np.zeros((pad, DT), np.float32)], axis=0)
        nbr_p = np.concatenate([nbr, np.zeros((pad, 3), np.int32)], axis=0)
    else:
        spatial_p, structural_p, nbr_p = spatial, structural, nbr

    sfull = _to_bf16(structural)

    # w1 rows match the a1T ordering: [structural; spatial; ones-row]
    w1t = np.asarray(W_comb, np.float32).T                       # [195, 256]
    w1 = _to_bf16(np.concatenate(
        [w1t[DS : DS + DT], w1t[0:DS],
         np.asarray(b_comb, np.float32)[None, :]], axis=0))      # [196, 256]
    w2 = _to_bf16(np.concatenate(
        [0.25 * np.asarray(W_agg, np.float32).T,
         np.asarray(b_agg, np.float32)[None, :]], axis=0))       # [132, 256]

    in_maps = []
    for c in range(ncores):
        sl = slice(c * npc, (c + 1) * npc)
        a1T = np.empty((KA, npc), np.float32)
        a1T[0:DT] = structural_p[sl].T
        a1T[DT : DT + DS] = spatial_p[sl].T
        a1T[DT + DS] = 1.0
        # idx[p, (g*subt + b)*3 + j] = nbr[c*npc + g*group + b*128 + p, j]
        ngt = npc // 128
        idx = np.ascontiguousarray(
            nbr_p[sl].reshape(ngt, 128, 3)
            .transpose(1, 0, 2).reshape(128, ngt * 3))
        in_maps.append({
            "a1T": _to_bf16(a1T),
            "sfull": sfull,
            "idx": idx,
            "w1": w1,
            "w2": w2,
        })
    return in_maps


_NC_CACHE = {}


def kernel(spatial, structural, neighbour, W_agg, b_agg, W_comb, b_comb):
    global last_exec_time_ns
    key = (NPC, N, GROUP)
    if key not in _NC_CACHE:
        _NC_CACHE[key] = build_nc(*key)
    nc = _NC_CACHE[key]

    in_maps = prep_inputs(
        spatial, structural, neighbour, W_agg, b_agg, W_comb, b_comb)

    trace = bool(int(os.environ.get("KERNEL_TRACE", "0")))
    tmpdir = os.environ.get("KERNEL_TMPDIR") or None
    res = run_bass_kernel_spmd(
        nc, in_maps, core_ids=list(range(NCORES)), trace=trace, tmpdir=tmpdir)
    last_exec_time_ns = res.exec_time_ns

    comb = np.concatenate(
        [np.asarray(r["out"], np.float32) for r in res.results], axis=1)[:, :N]
    out1 = np.ascontiguousarray(comb[:DO, :].T)
    out2 = np.ascontiguousarray(comb[DO:, :].T)
    return out1, out2
